# revision 39
# baseline (speedup 1.0000x reference)
"""Trainium2 Bass kernel for a YOLO-style detection loss.

Reference semantics (per image b):
  cls BCE-with-logits vs a one-hot scatter target at object centers:
    pos_cls = sum_b( sum_{unique pos p} (softplus(x_p) - x_p) / n_pos_b ) / (B*N)
    neg_cls = sum_b( (sum_all softplus(x) - sum_{unique p} softplus(x_p))
                     / (C*H*W - n_pos_b) ) / B
  bbox L1 at object centers (all N objects, duplicates included):
    bbox = sum_{b,n} mean_4 |bbox_pred[b,:,gy,gx] - tgt| / (B*N)
  out = [0.5*cls + 7.5*bbox + 1e-6, cls, bbox],  cls = pos_cls + 0.25*neg_cls

Sharding: data-parallel over batch, 2 images per core across 8 cores.  Every
term of the output is linear in per-core partial sums, so each core emits its
fully-normalized *contribution* to the [3]-vector (with eps/8 so the constant
also sums correctly) and the host unshard is a pure elementwise sum.

Per core the kernel streams its 16.4 MB cls_pred shard once (DMA-bound).
This compiler's ACT tables have no Softplus, and two full ACT passes
(Exp + Ln) would exceed the DMA time, so softplus sums use a pairing trick:
  ln(1+a) + ln(1+b) = ln(1 + (a+b+ab))
One Exp pass produces u = e^x for all elements, the otherwise-idle DVE folds
element pairs into v = u1+u2+u1*u2, and the Ln(1+v) accumulation pass runs on
only half the elements: ACT ~42us, DVE ~34us, DMA ~46us -> DMA-bound.
(x in [-6, 6] here, so e^x in [3e-3, 4e2] and no range reduction is needed.)

The object-center terms use ~256 indirectly-gathered values; bbox_pred is
never streamed.  Duplicate (label,gy,gx) scatter targets are deduped with a
64x64 key-equality matrix (PE transpose + DVE compare + lower-triangle mask).
"""

import os

import numpy as np

# ---- problem constants (hardcoded per contract) ----
B, C, H, W, N = 16, 80, 160, 160, 32
N_CORES = 8
BS = B // N_CORES          # images per core = 2
CHW = C * H * W            # 2_048_000
HW = H * W                 # 25_600
NOBJ = BS * N              # 64 objects per core
CLS_FLAT = BS * CHW        # 4_096_000
BB_FLAT = BS * 4 * HW      # 204_800
P = 128
FREE = CLS_FLAT // P       # 32_000
# Descending chunk sizes: steady-state chunks are large (amortize the ~352-cycle
# ACT instruction overhead), the last chunks are small so the post-DMA tail
# chain (Exp -> DVE pair-combine x2 -> Ln) is short.
CHUNKS = [1500, 2500, 4000, 5000, 5000, 5000, 4000, 3000, 2000]
if os.environ.get("BASS_CHUNKS"):  # dev-only sweep hook
    CHUNKS = [int(x) for x in os.environ["BASS_CHUNKS"].split(",")]
assert sum(CHUNKS) == FREE and all(c % 4 == 0 for c in CHUNKS)
# Chunks that get a second pairing level (one extra DVE tensor_mul; Ln then
# runs over ch/4 instead of ch/2), dropping ACT busy-time below the DMA
# floor.  The last chunks stay shallow so the post-DMA tail chain is short.
LEVEL2 = set(range(len(CHUNKS) - 2))
if os.environ.get("BASS_LEVEL2"):  # dev-only sweep hook
    LEVEL2 = {int(x) for x in os.environ["BASS_LEVEL2"].split(",") if x != ""}
LN_LAG = 1  # how many chunks the Ln pass trails the Exp pass
ROWS_PER_IMG = CHW // FREE  # 64 partitions per image

_cache = {}


def _build_nc():
    import concourse.bacc as bacc
    import concourse.bass as bass
    import concourse.mybir as mybir
    import concourse.tile as tile
    from concourse.masks import make_identity

    dt = mybir.dt
    f32 = dt.float32
    i32 = dt.int32
    Alu = mybir.AluOpType
    Act = mybir.ActivationFunctionType

    nc = bacc.Bacc(
        "TRN2",
        target_bir_lowering=False,
        debug=False,
        enable_asserts=False,
        num_devices=N_CORES,
    )

    cls_d = nc.dram_tensor("cls", [CLS_FLAT], f32, kind="ExternalInput")
    bb_d = nc.dram_tensor("bb", [BB_FLAT], f32, kind="ExternalInput")
    gt_d = nc.dram_tensor("gt", [NOBJ, 4], f32, kind="ExternalInput")
    lbl_d = nc.dram_tensor("lbl", [NOBJ, 1], i32, kind="ExternalInput")
    out_d = nc.dram_tensor("out", [1, 3], f32, kind="ExternalOutput")

    cls2d = cls_d.ap().rearrange("(p m) -> p m", p=P)           # [128, 32000]
    cls_rows = cls_d.ap().rearrange("(n o) -> n o", o=1)        # [4096000, 1]
    bb_rows = bb_d.ap().rearrange("(n o) -> n o", o=1)          # [204800, 1]

    def softplus(out_ap, in_ap, accum=None, tmp=None):
        # No Softplus act-table on this stack: ln(1 + e^x) in two passes.
        t = tmp if tmp is not None else out_ap
        nc.scalar.activation(t, in_ap, Act.Exp)
        nc.scalar.activation(out_ap, t, Act.Ln, bias=1.0, accum_out=accum)

    with tile.TileContext(nc) as tc:
        with (
            tc.tile_pool(name="const", bufs=1) as cpool,
            tc.tile_pool(name="small", bufs=1) as spool,
            tc.tile_pool(name="big", bufs=5) as bpool,
            tc.tile_pool(name="pair", bufs=4) as vpool,
            tc.tile_pool(name="psum", bufs=2, space="PSUM") as ppool,
        ):
            # ---------------- constants ----------------
            ident = cpool.tile([NOBJ, NOBJ], f32)
            make_identity(nc, ident[:])

            bsel64 = cpool.tile([NOBJ, 2], f32)       # object -> image selector
            nc.gpsimd.memset(bsel64[:], 0.0)
            nc.gpsimd.memset(bsel64[0:N, 0:1], 1.0)
            nc.gpsimd.memset(bsel64[N : 2 * N, 1:2], 1.0)

            bsel128 = cpool.tile([P, 2], f32)         # partition -> image selector
            nc.gpsimd.memset(bsel128[:], 0.0)
            nc.gpsimd.memset(bsel128[0:ROWS_PER_IMG, 0:1], 1.0)
            nc.gpsimd.memset(bsel128[ROWS_PER_IMG:P, 1:2], 1.0)

            boff_cls = cpool.tile([NOBJ, 1], f32)     # per-object image offset in cls
            nc.gpsimd.memset(boff_cls[0:N, :], 0.0)
            nc.gpsimd.memset(boff_cls[N : 2 * N, :], float(CHW))

            boff_bb = cpool.tile([NOBJ, 1], f32)      # per-object image offset in bbox
            nc.gpsimd.memset(boff_bb[0:N, :], 0.0)
            nc.gpsimd.memset(boff_bb[N : 2 * N, :], float(4 * HW))

            cmul_i = cpool.tile([NOBJ, 4], i32)       # [0, HW, 2HW, 3HW] per row
            nc.gpsimd.iota(cmul_i[:], pattern=[[HW, 4]], channel_multiplier=0)
            cmul_f = cpool.tile([NOBJ, 4], f32)
            nc.vector.tensor_copy(cmul_f[:], cmul_i[:])

            ones2 = cpool.tile([2, 1], f32)
            nc.gpsimd.memset(ones2[:], 1.0)

            # ---------------- tiny input loads ----------------
            g = spool.tile([NOBJ, 4], f32)
            nc.sync.dma_start(g[:], gt_d[:])
            li = spool.tile([NOBJ, 1], i32)
            nc.sync.dma_start(li[:], lbl_d[:])
            lf = spool.tile([NOBJ, 1], f32)
            nc.vector.tensor_copy(lf[:], li[:])

            # ---------------- object centers ----------------
            # T = gt * W  (x1,y1,x2,y2 in feature coords; H == W == 160)
            T = spool.tile([NOBJ, 4], f32)
            nc.vector.tensor_scalar_mul(T[:], g[:], float(W))

            cxy = spool.tile([NOBJ, 2], f32)          # [cx, cy] pre-clip sums
            nc.vector.tensor_tensor(
                out=cxy[:, 0:1], in0=T[:, 0:1], in1=T[:, 2:3], op=Alu.add
            )
            nc.vector.tensor_tensor(
                out=cxy[:, 1:2], in0=T[:, 1:2], in1=T[:, 3:4], op=Alu.add
            )
            cxy2 = spool.tile([NOBJ, 2], f32)
            nc.vector.tensor_scalar(
                out=cxy2[:], in0=cxy[:], scalar1=0.5, scalar2=0.0,
                op0=Alu.mult, op1=Alu.max,
            )
            cxy3 = spool.tile([NOBJ, 2], f32)
            nc.vector.tensor_scalar_min(cxy3[:], cxy2[:], float(W - 1))

            # floor() robust to the convert's rounding mode: conv to int,
            # back to float, subtract 1 where the roundtrip overshot.
            cint = spool.tile([NOBJ, 2], i32)
            nc.vector.tensor_copy(cint[:], cxy3[:])
            cif = spool.tile([NOBJ, 2], f32)
            nc.vector.tensor_copy(cif[:], cint[:])
            cgt = spool.tile([NOBJ, 2], f32)
            nc.vector.tensor_tensor(out=cgt[:], in0=cif[:], in1=cxy3[:], op=Alu.is_gt)
            gxy = spool.tile([NOBJ, 2], f32)          # [gx, gy] floored, exact ints
            nc.vector.tensor_sub(gxy[:], cif[:], cgt[:])

            # ---------------- gather offsets ----------------
            rowoff = spool.tile([NOBJ, 1], f32)       # gy*W + gx
            nc.vector.scalar_tensor_tensor(
                out=rowoff[:], in0=gxy[:, 1:2], scalar=float(W),
                in1=gxy[:, 0:1], op0=Alu.mult, op1=Alu.add,
            )
            key = spool.tile([NOBJ, 1], f32)          # b*CHW + lbl*HW + gy*W + gx
            nc.vector.scalar_tensor_tensor(
                out=key[:], in0=lf[:], scalar=float(HW),
                in1=rowoff[:], op0=Alu.mult, op1=Alu.add,
            )
            key2 = spool.tile([NOBJ, 1], f32)
            nc.vector.tensor_add(key2[:], key[:], boff_cls[:])
            idx_cls = spool.tile([NOBJ, 1], i32)
            nc.vector.tensor_copy(idx_cls[:], key2[:])

            bbbase = spool.tile([NOBJ, 1], f32)       # b*4*HW + gy*W + gx
            nc.vector.tensor_add(bbbase[:], rowoff[:], boff_bb[:])
            bb4f = spool.tile([NOBJ, 4], f32)
            nc.vector.tensor_scalar(
                out=bb4f[:], in0=cmul_f[:], scalar1=bbbase[:], scalar2=None,
                op0=Alu.add,
            )
            bb4i = spool.tile([NOBJ, 4], i32)
            nc.vector.tensor_copy(bb4i[:], bb4f[:])

            # ---------------- indirect gathers ----------------
            xp = spool.tile([NOBJ, 1], f32)           # cls_pred at pos targets
            nc.gpsimd.indirect_dma_start(
                out=xp[:], out_offset=None, in_=cls_rows,
                in_offset=bass.IndirectOffsetOnAxis(ap=idx_cls[:, 0:1], axis=0),
            )
            bbp = spool.tile([NOBJ, 4], f32)          # bbox_pred at centers
            for c in range(4):
                nc.gpsimd.indirect_dma_start(
                    out=bbp[:, c : c + 1], out_offset=None, in_=bb_rows,
                    in_offset=bass.IndirectOffsetOnAxis(
                        ap=bb4i[:, c : c + 1], axis=0
                    ),
                )

            # ---------------- dedupe scatter collisions ----------------
            kT = ppool.tile([NOBJ, NOBJ], f32, space="PSUM")
            nc.tensor.transpose(
                out=kT[:], in_=key2[:].to_broadcast([NOBJ, NOBJ]), identity=ident[:]
            )
            eq = spool.tile([NOBJ, NOBJ], f32)
            nc.vector.tensor_tensor(
                out=eq[:], in0=key2[:].to_broadcast([NOBJ, NOBJ]), in1=kT[:],
                op=Alu.is_equal,
            )
            eqm = spool.tile([NOBJ, NOBJ], f32)       # keep strictly-lower (j < i)
            nc.gpsimd.affine_select(
                out=eqm[:], in_=eq[:], base=-1, channel_multiplier=1,
                pattern=[[-1, NOBJ]], compare_op=Alu.is_ge, fill=0.0,
            )
            dup = spool.tile([NOBJ, 1], f32)
            nc.vector.tensor_reduce(
                dup[:], eqm[:], axis=mybir.AxisListType.X, op=Alu.max
            )
            keep = spool.tile([NOBJ, 1], f32)         # 1 - dup
            nc.vector.tensor_scalar(
                out=keep[:], in0=dup[:], scalar1=-1.0, scalar2=1.0,
                op0=Alu.mult, op1=Alu.add,
            )

            # ---------------- big stream: sum softplus(cls_pred) ----------------
            # u = e^x (ACT, all elems); v = u1+u2+u1*u2 (DVE, pairs);
            # ln(1+v) + row-accumulate (ACT, half the elems).
            # ACT program order is software-pipelined (Exp0, Exp1, Ln0, Exp2,
            # Ln1, ...) so a chunk's Ln never stalls the next chunk's Exp.
            acc = spool.tile([P, len(CHUNKS)], f32)
            off = 0
            pending = []  # (w_tile, chunk_idx) awaiting the Ln pass

            def emit_ln(w, k, bias=0.0):
                # bias=0: w holds z-values (products of 1+e^x); bias=1: raw u.
                nc.scalar.activation(
                    w[:], w[:], Act.Ln, bias=bias, accum_out=acc[:, k : k + 1]
                )

            def pair_combine_z(dst, a, b):
                # dst = (1+a)(1+b), built as ((a+1)*b) + 1 + a in two ops;
                # carrying z := 1+v makes deeper levels a single multiply.
                nc.vector.scalar_tensor_tensor(
                    out=dst, in0=a, scalar=1.0, in1=b, op0=Alu.add, op1=Alu.mult
                )
                nc.vector.scalar_tensor_tensor(
                    out=dst, in0=dst, scalar=1.0, in1=a, op0=Alu.add, op1=Alu.add
                )

            for k, ch in enumerate(CHUNKS):
                t = bpool.tile([P, ch], f32, tag="big")
                nc.sync.dma_start(t[:], cls2d[:, off : off + ch])
                nc.scalar.activation(t[:], t[:], Act.Exp)
                if len(pending) >= LN_LAG:
                    emit_ln(*pending.pop(0))
                w = vpool.tile([P, ch // 2], f32, tag="pair")
                pair_combine_z(w[:], t[:, : ch // 2], t[:, ch // 2 :])
                if k in LEVEL2:
                    y = vpool.tile([P, ch // 4], f32, tag="pair2")
                    nc.gpsimd.tensor_mul(y[:], w[:, : ch // 4], w[:, ch // 4 :])
                    pending.append((y, k))
                else:
                    pending.append((w, k))
                off += ch
            for item in pending:
                emit_ln(*item)

            # ---------------- per-object terms ----------------
            # Everything consuming the gathered values is pinned late
            # (tile_wait_until) so no engine's in-order stream blocks on the
            # tiny gather DMAs before the big streaming work.
            with tc.tile_wait_until(0.06):
                sp = spool.tile([NOBJ, 1], f32)       # softplus(x_p)
                sptmp = spool.tile([NOBJ, 1], f32)
                softplus(sp[:], xp[:], tmp=sptmp[:])
                bce = spool.tile([NOBJ, 1], f32)      # positive-class bce
                nc.vector.tensor_sub(bce[:], sp[:], xp[:])

                d4 = spool.tile([NOBJ, 4], f32)
                nc.vector.tensor_sub(d4[:], bbp[:], T[:])
                l1r = spool.tile([NOBJ, 1], f32)
                nc.vector.tensor_reduce(
                    l1r[:], d4[:], axis=mybir.AxisListType.X, op=Alu.add,
                    apply_absolute_value=True,
                )

                M = spool.tile([NOBJ, 4], f32)  # [keep*bce, keep*sp, keep, l1/4]
                nc.vector.tensor_mul(M[:, 0:1], bce[:], keep[:])
                nc.vector.tensor_mul(M[:, 1:2], sp[:], keep[:])
                nc.vector.tensor_copy(M[:, 2:3], keep[:])
                nc.vector.tensor_scalar_mul(M[:, 3:4], l1r[:], 0.25)

                Fp = ppool.tile([2, 4], f32, space="PSUM")
                nc.tensor.matmul(Fp[:], bsel64[:], M[:], start=True, stop=True)
                F = spool.tile([2, 4], f32)           # per-image A, C, cnt, bbox
                nc.vector.tensor_copy(F[:], Fp[:])

            s0col = spool.tile([P, 1], f32)
            nc.vector.tensor_reduce(
                s0col[:], acc[:], axis=mybir.AxisListType.X, op=Alu.add
            )
            S0p = ppool.tile([2, 1], f32, space="PSUM")
            nc.tensor.matmul(S0p[:], bsel128[:], s0col[:], start=True, stop=True)

            # ---------------- per-image normalization ----------------
            negsum = spool.tile([2, 1], f32)          # S0 - sum(keep*sp)
            nc.vector.tensor_sub(negsum[:], S0p[:], F[:, 1:2])
            negcnt = spool.tile([2, 1], f32)          # CHW - cnt
            nc.vector.tensor_scalar(
                out=negcnt[:], in0=F[:, 2:3], scalar1=-1.0, scalar2=float(CHW),
                op0=Alu.mult, op1=Alu.add,
            )
            rc = spool.tile([2, 1], f32)
            nc.vector.reciprocal(rc[:], F[:, 2:3])
            rn = spool.tile([2, 1], f32)
            nc.vector.reciprocal(rn[:], negcnt[:])

            G = spool.tile([2, 3], f32)  # [pos_ratio, neg_ratio, bbox_sum]
            nc.vector.tensor_mul(G[:, 0:1], F[:, 0:1], rc[:])
            nc.vector.tensor_mul(G[:, 1:2], negsum[:], rn[:])
            nc.vector.tensor_copy(G[:, 2:3], F[:, 3:4])

            Rp = ppool.tile([1, 3], f32, space="PSUM")
            nc.tensor.matmul(Rp[:], ones2[:], G[:], start=True, stop=True)
            r = spool.tile([1, 3], f32)
            nc.vector.tensor_copy(r[:], Rp[:])

            # ---------------- final scalars (core's contribution) ----------------
            o = spool.tile([1, 3], f32)
            t2 = spool.tile([1, 1], f32)
            nc.vector.tensor_scalar_mul(t2[:], r[:, 1:2], 1.0 / (4 * B))
            # cls_i = pos_ratio/ (B*N) + neg_ratio/(4*B)
            nc.vector.scalar_tensor_tensor(
                out=o[:, 1:2], in0=r[:, 0:1], scalar=1.0 / (B * N),
                in1=t2[:], op0=Alu.mult, op1=Alu.add,
            )
            nc.vector.tensor_scalar_mul(o[:, 2:3], r[:, 2:3], 1.0 / (B * N))
            t3 = spool.tile([1, 1], f32)
            nc.vector.tensor_scalar(
                out=t3[:], in0=o[:, 2:3], scalar1=7.5, scalar2=1e-6 / N_CORES,
                op0=Alu.mult, op1=Alu.add,
            )
            nc.vector.scalar_tensor_tensor(
                out=o[:, 0:1], in0=o[:, 1:2], scalar=0.5,
                in1=t3[:], op0=Alu.mult, op1=Alu.add,
            )
            nc.sync.dma_start(out_d[:], o[:])

    # Both Exp and Ln must resolve to the one table set that contains them
    # both ("natural_log_exp_and_others"); otherwise the table-load pass picks
    # different sets and inserts a ~2.7us table reload between every Exp and
    # Ln, each contending with the streaming DMAs.  Hide Exp/Ln from every
    # other set (set indices are preserved, so act_func_set_id stays valid).
    import concourse.bacc as bacc_mod

    real_get_tables = bacc_mod.get_activation_tables

    def one_table(arch):
        tables = real_get_tables(arch)
        for name, s in tables.items():
            if name != "natural_log_exp_and_others":
                s.discard(Act.Exp)
                s.discard(Act.Ln)
        return tables

    bacc_mod.get_activation_tables = one_table
    try:
        nc.compile()
    finally:
        bacc_mod.get_activation_tables = real_get_tables
    return nc


def _get_nc():
    if "nc" not in _cache:
        _cache["nc"] = _build_nc()
    return _cache["nc"]


def _make_in_maps(cls_pred, bbox_pred, gt_bboxes, gt_labels):
    cls_pred = np.ascontiguousarray(np.asarray(cls_pred, dtype=np.float32))
    bbox_pred = np.ascontiguousarray(np.asarray(bbox_pred, dtype=np.float32))
    gt_bboxes = np.ascontiguousarray(np.asarray(gt_bboxes, dtype=np.float32))
    gt_labels = np.asarray(gt_labels).astype(np.int32)
    in_maps = []
    for i in range(N_CORES):
        s = slice(i * BS, (i + 1) * BS)
        in_maps.append(
            {
                "cls": cls_pred[s].reshape(CLS_FLAT),
                "bb": bbox_pred[s].reshape(BB_FLAT),
                "gt": gt_bboxes[s].reshape(NOBJ, 4),
                "lbl": np.ascontiguousarray(gt_labels[s].reshape(NOBJ, 1)),
            }
        )
    return in_maps


def kernel_with_results(trace=False, **inputs):
    from concourse.bass_utils import run_bass_kernel_spmd

    nc = _get_nc()
    in_maps = _make_in_maps(**inputs)
    res = run_bass_kernel_spmd(
        nc, in_maps, core_ids=list(range(N_CORES)), trace=trace
    )
    total = np.zeros(3, dtype=np.float64)
    for core in res.results:
        total += core["out"].reshape(3).astype(np.float64)
    return total.astype(np.float32), res


def kernel(**inputs):
    out, _ = kernel_with_results(**inputs)
    return out


# revision 44
# speedup vs baseline: 85.7050x; 85.7050x over previous
"""Trainium2 Bass kernel for a YOLO-style detection loss.

Reference semantics (per image b):
  cls BCE-with-logits vs a one-hot scatter target at object centers:
    pos_cls = sum_b( sum_{unique pos p} (softplus(x_p) - x_p) / n_pos_b ) / (B*N)
    neg_cls = sum_b( (sum_all softplus(x) - sum_{unique p} softplus(x_p))
                     / (C*H*W - n_pos_b) ) / B
  bbox L1 at object centers (all N objects, duplicates included):
    bbox = sum_{b,n} mean_4 |bbox_pred[b,:,gy,gx] - tgt| / (B*N)
  out = [0.5*cls + 7.5*bbox + 1e-6, cls, bbox],  cls = pos_cls + 0.25*neg_cls

Sharding: data-parallel over batch, 2 images per core across 8 cores.  Every
term of the output is linear in per-core partial sums, so each core emits its
fully-normalized *contribution* to the [3]-vector (with eps/8 so the constant
also sums correctly) and the host unshard is a pure elementwise sum.

Per core the kernel streams its 16.4 MB cls_pred shard once (DMA-bound).
This compiler's ACT tables have no Softplus, and two full ACT passes
(Exp + Ln) would exceed the DMA time, so softplus sums use a pairing trick:
  ln(1+a) + ln(1+b) = ln(1 + (a+b+ab))
One Exp pass produces u = e^x for all elements, the otherwise-idle DVE folds
element pairs into v = u1+u2+u1*u2, and the Ln(1+v) accumulation pass runs on
only half the elements: ACT ~42us, DVE ~34us, DMA ~46us -> DMA-bound.
(x in [-6, 6] here, so e^x in [3e-3, 4e2] and no range reduction is needed.)

The object-center terms use ~256 indirectly-gathered values; bbox_pred is
never streamed.  Duplicate (label,gy,gx) scatter targets are deduped with a
64x64 key-equality matrix (PE transpose + DVE compare + lower-triangle mask).
"""

import os

import numpy as np

# ---- problem constants (hardcoded per contract) ----
B, C, H, W, N = 16, 80, 160, 160, 32
N_CORES = 8
BS = B // N_CORES          # images per core = 2
CHW = C * H * W            # 2_048_000
HW = H * W                 # 25_600
NOBJ = BS * N              # 64 objects per core
CLS_FLAT = BS * CHW        # 4_096_000
BB_FLAT = BS * 4 * HW      # 204_800
P = 128
FREE = CLS_FLAT // P       # 32_000
# Descending chunk sizes: steady-state chunks are large (amortize the ~352-cycle
# ACT instruction overhead), the last chunks are small so the post-DMA tail
# chain (Exp -> DVE pair-combine x2 -> Ln) is short.
CHUNKS = [1500, 2500, 4000, 5000, 5000, 5000, 4000, 3000, 2000]
if os.environ.get("BASS_CHUNKS"):  # dev-only sweep hook
    CHUNKS = [int(x) for x in os.environ["BASS_CHUNKS"].split(",")]
assert sum(CHUNKS) == FREE and all(c % 4 == 0 for c in CHUNKS)
# Chunks that get a second pairing level (one extra DVE tensor_mul; Ln then
# runs over ch/4 instead of ch/2), dropping ACT busy-time below the DMA
# floor.  The last chunks stay shallow so the post-DMA tail chain is short.
LEVEL2 = set(range(len(CHUNKS) - 2))
if os.environ.get("BASS_LEVEL2"):  # dev-only sweep hook
    LEVEL2 = {int(x) for x in os.environ["BASS_LEVEL2"].split(",") if x != ""}
LN_LAG = 1  # how many chunks the Ln pass trails the Exp pass
ROWS_PER_IMG = CHW // FREE  # 64 partitions per image

_cache = {}


def _build_nc(repeat=1):
    import concourse.bacc as bacc
    import concourse.bass as bass
    import concourse.mybir as mybir
    import concourse.tile as tile
    from concourse.masks import make_identity

    dt = mybir.dt
    f32 = dt.float32
    i32 = dt.int32
    Alu = mybir.AluOpType
    Act = mybir.ActivationFunctionType

    nc = bacc.Bacc(
        "TRN2",
        target_bir_lowering=False,
        debug=False,
        enable_asserts=False,
        num_devices=N_CORES,
    )

    cls_d = nc.dram_tensor("cls", [CLS_FLAT], f32, kind="ExternalInput")
    bb_d = nc.dram_tensor("bb", [BB_FLAT], f32, kind="ExternalInput")
    gt_d = nc.dram_tensor("gt", [NOBJ, 4], f32, kind="ExternalInput")
    lbl_d = nc.dram_tensor("lbl", [NOBJ, 1], i32, kind="ExternalInput")
    out_d = nc.dram_tensor("out", [1, 3], f32, kind="ExternalOutput")

    cls2d = cls_d.ap().rearrange("(p m) -> p m", p=P)           # [128, 32000]
    cls_rows = cls_d.ap().rearrange("(n o) -> n o", o=1)        # [4096000, 1]
    bb_rows = bb_d.ap().rearrange("(n o) -> n o", o=1)          # [204800, 1]

    def softplus(out_ap, in_ap, accum=None, tmp=None):
        # No Softplus act-table on this stack: ln(1 + e^x) in two passes.
        t = tmp if tmp is not None else out_ap
        nc.scalar.activation(t, in_ap, Act.Exp)
        nc.scalar.activation(out_ap, t, Act.Ln, bias=1.0, accum_out=accum)

    with tile.TileContext(nc) as tc:
        with (
            tc.tile_pool(name="const", bufs=1) as cpool,
            tc.tile_pool(name="small", bufs=1) as spool,
            tc.tile_pool(name="big", bufs=5) as bpool,
            tc.tile_pool(name="pair", bufs=4) as vpool,
            tc.tile_pool(name="psum", bufs=2, space="PSUM") as ppool,
        ):
            # ---------------- constants ----------------
            ident = cpool.tile([NOBJ, NOBJ], f32)
            make_identity(nc, ident[:])

            bsel64 = cpool.tile([NOBJ, 2], f32)       # object -> image selector
            nc.gpsimd.memset(bsel64[:], 0.0)
            nc.gpsimd.memset(bsel64[0:N, 0:1], 1.0)
            nc.gpsimd.memset(bsel64[N : 2 * N, 1:2], 1.0)

            bsel128 = cpool.tile([P, 2], f32)         # partition -> image selector
            nc.gpsimd.memset(bsel128[:], 0.0)
            nc.gpsimd.memset(bsel128[0:ROWS_PER_IMG, 0:1], 1.0)
            nc.gpsimd.memset(bsel128[ROWS_PER_IMG:P, 1:2], 1.0)

            boff_cls = cpool.tile([NOBJ, 1], f32)     # per-object image offset in cls
            nc.gpsimd.memset(boff_cls[0:N, :], 0.0)
            nc.gpsimd.memset(boff_cls[N : 2 * N, :], float(CHW))

            boff_bb = cpool.tile([NOBJ, 1], f32)      # per-object image offset in bbox
            nc.gpsimd.memset(boff_bb[0:N, :], 0.0)
            nc.gpsimd.memset(boff_bb[N : 2 * N, :], float(4 * HW))

            cmul_i = cpool.tile([NOBJ, 4], i32)       # [0, HW, 2HW, 3HW] per row
            nc.gpsimd.iota(cmul_i[:], pattern=[[HW, 4]], channel_multiplier=0)
            cmul_f = cpool.tile([NOBJ, 4], f32)
            nc.vector.tensor_copy(cmul_f[:], cmul_i[:])

            ones2 = cpool.tile([2, 1], f32)
            nc.gpsimd.memset(ones2[:], 1.0)

            # repeat>1 loops the whole body for slope-based device timing
            # (same tile tags across iterations -> slots reused, iterations
            # serialize through the SBUF WAR deps, which is what we want).
            for _rep in range(repeat):
                _body(nc, tc, spool, bpool, vpool, ppool, mybir, bass, Alu, Act,
                      softplus, cls2d, cls_rows, bb_rows, gt_d, lbl_d, out_d,
                      ident, bsel64, bsel128, boff_cls, boff_bb, cmul_f, ones2)

    # Both Exp and Ln must resolve to the one table set that contains them
    # both ("natural_log_exp_and_others"); otherwise the table-load pass picks
    # different sets and inserts a ~2.7us table reload between every Exp and
    # Ln, each contending with the streaming DMAs.  Hide Exp/Ln from every
    # other set (set indices are preserved, so act_func_set_id stays valid).
    import concourse.bacc as bacc_mod

    real_get_tables = bacc_mod.get_activation_tables

    def one_table(arch):
        tables = real_get_tables(arch)
        for name, s in tables.items():
            if name != "natural_log_exp_and_others":
                s.discard(Act.Exp)
                s.discard(Act.Ln)
        return tables

    bacc_mod.get_activation_tables = one_table
    try:
        nc.compile()
    finally:
        bacc_mod.get_activation_tables = real_get_tables
    return nc


def _body(nc, tc, spool, bpool, vpool, ppool, mybir, bass, Alu, Act, softplus,
          cls2d, cls_rows, bb_rows, gt_d, lbl_d, out_d,
          ident, bsel64, bsel128, boff_cls, boff_bb, cmul_f, ones2):
    dt = mybir.dt
    f32 = dt.float32
    i32 = dt.int32
    if True:
        if True:
            # ---------------- tiny input loads ----------------
            g = spool.tile([NOBJ, 4], f32)
            nc.sync.dma_start(g[:], gt_d[:])
            li = spool.tile([NOBJ, 1], i32)
            nc.sync.dma_start(li[:], lbl_d[:])
            lf = spool.tile([NOBJ, 1], f32)
            nc.vector.tensor_copy(lf[:], li[:])

            # ---------------- object centers ----------------
            # T = gt * W  (x1,y1,x2,y2 in feature coords; H == W == 160)
            T = spool.tile([NOBJ, 4], f32)
            nc.vector.tensor_scalar_mul(T[:], g[:], float(W))

            cxy = spool.tile([NOBJ, 2], f32)          # [cx, cy] pre-clip sums
            nc.vector.tensor_tensor(
                out=cxy[:, 0:1], in0=T[:, 0:1], in1=T[:, 2:3], op=Alu.add
            )
            nc.vector.tensor_tensor(
                out=cxy[:, 1:2], in0=T[:, 1:2], in1=T[:, 3:4], op=Alu.add
            )
            cxy2 = spool.tile([NOBJ, 2], f32)
            nc.vector.tensor_scalar(
                out=cxy2[:], in0=cxy[:], scalar1=0.5, scalar2=0.0,
                op0=Alu.mult, op1=Alu.max,
            )
            cxy3 = spool.tile([NOBJ, 2], f32)
            nc.vector.tensor_scalar_min(cxy3[:], cxy2[:], float(W - 1))

            # floor() robust to the convert's rounding mode: conv to int,
            # back to float, subtract 1 where the roundtrip overshot.
            cint = spool.tile([NOBJ, 2], i32)
            nc.vector.tensor_copy(cint[:], cxy3[:])
            cif = spool.tile([NOBJ, 2], f32)
            nc.vector.tensor_copy(cif[:], cint[:])
            cgt = spool.tile([NOBJ, 2], f32)
            nc.vector.tensor_tensor(out=cgt[:], in0=cif[:], in1=cxy3[:], op=Alu.is_gt)
            gxy = spool.tile([NOBJ, 2], f32)          # [gx, gy] floored, exact ints
            nc.vector.tensor_sub(gxy[:], cif[:], cgt[:])

            # ---------------- gather offsets ----------------
            rowoff = spool.tile([NOBJ, 1], f32)       # gy*W + gx
            nc.vector.scalar_tensor_tensor(
                out=rowoff[:], in0=gxy[:, 1:2], scalar=float(W),
                in1=gxy[:, 0:1], op0=Alu.mult, op1=Alu.add,
            )
            key = spool.tile([NOBJ, 1], f32)          # b*CHW + lbl*HW + gy*W + gx
            nc.vector.scalar_tensor_tensor(
                out=key[:], in0=lf[:], scalar=float(HW),
                in1=rowoff[:], op0=Alu.mult, op1=Alu.add,
            )
            key2 = spool.tile([NOBJ, 1], f32)
            nc.vector.tensor_add(key2[:], key[:], boff_cls[:])
            idx_cls = spool.tile([NOBJ, 1], i32)
            nc.vector.tensor_copy(idx_cls[:], key2[:])

            bbbase = spool.tile([NOBJ, 1], f32)       # b*4*HW + gy*W + gx
            nc.vector.tensor_add(bbbase[:], rowoff[:], boff_bb[:])
            bb4f = spool.tile([NOBJ, 4], f32)
            nc.vector.tensor_scalar(
                out=bb4f[:], in0=cmul_f[:], scalar1=bbbase[:], scalar2=None,
                op0=Alu.add,
            )
            bb4i = spool.tile([NOBJ, 4], i32)
            nc.vector.tensor_copy(bb4i[:], bb4f[:])

            # ---------------- indirect gathers ----------------
            xp = spool.tile([NOBJ, 1], f32)           # cls_pred at pos targets
            nc.gpsimd.indirect_dma_start(
                out=xp[:], out_offset=None, in_=cls_rows,
                in_offset=bass.IndirectOffsetOnAxis(ap=idx_cls[:, 0:1], axis=0),
            )
            bbp = spool.tile([NOBJ, 4], f32)          # bbox_pred at centers
            for c in range(4):
                nc.gpsimd.indirect_dma_start(
                    out=bbp[:, c : c + 1], out_offset=None, in_=bb_rows,
                    in_offset=bass.IndirectOffsetOnAxis(
                        ap=bb4i[:, c : c + 1], axis=0
                    ),
                )

            # ---------------- dedupe scatter collisions ----------------
            kT = ppool.tile([NOBJ, NOBJ], f32, space="PSUM")
            nc.tensor.transpose(
                out=kT[:], in_=key2[:].to_broadcast([NOBJ, NOBJ]), identity=ident[:]
            )
            eq = spool.tile([NOBJ, NOBJ], f32)
            nc.vector.tensor_tensor(
                out=eq[:], in0=key2[:].to_broadcast([NOBJ, NOBJ]), in1=kT[:],
                op=Alu.is_equal,
            )
            eqm = spool.tile([NOBJ, NOBJ], f32)       # keep strictly-lower (j < i)
            nc.gpsimd.affine_select(
                out=eqm[:], in_=eq[:], base=-1, channel_multiplier=1,
                pattern=[[-1, NOBJ]], compare_op=Alu.is_ge, fill=0.0,
            )
            dup = spool.tile([NOBJ, 1], f32)
            nc.vector.tensor_reduce(
                dup[:], eqm[:], axis=mybir.AxisListType.X, op=Alu.max
            )
            keep = spool.tile([NOBJ, 1], f32)         # 1 - dup
            nc.vector.tensor_scalar(
                out=keep[:], in0=dup[:], scalar1=-1.0, scalar2=1.0,
                op0=Alu.mult, op1=Alu.add,
            )

            # ---------------- big stream: sum softplus(cls_pred) ----------------
            # u = e^x (ACT, all elems); v = u1+u2+u1*u2 (DVE, pairs);
            # ln(1+v) + row-accumulate (ACT, half the elems).
            # ACT program order is software-pipelined (Exp0, Exp1, Ln0, Exp2,
            # Ln1, ...) so a chunk's Ln never stalls the next chunk's Exp.
            acc = spool.tile([P, len(CHUNKS)], f32)
            off = 0
            pending = []  # (w_tile, chunk_idx) awaiting the Ln pass

            def emit_ln(w, k, bias=0.0):
                # bias=0: w holds z-values (products of 1+e^x); bias=1: raw u.
                nc.scalar.activation(
                    w[:], w[:], Act.Ln, bias=bias, accum_out=acc[:, k : k + 1]
                )

            def pair_combine_z(dst, a, b):
                # dst = (1+a)(1+b), built as ((a+1)*b) + 1 + a in two ops;
                # carrying z := 1+v makes deeper levels a single multiply.
                nc.vector.scalar_tensor_tensor(
                    out=dst, in0=a, scalar=1.0, in1=b, op0=Alu.add, op1=Alu.mult
                )
                nc.vector.scalar_tensor_tensor(
                    out=dst, in0=dst, scalar=1.0, in1=a, op0=Alu.add, op1=Alu.add
                )

            for k, ch in enumerate(CHUNKS):
                t = bpool.tile([P, ch], f32, tag="big")
                nc.sync.dma_start(t[:], cls2d[:, off : off + ch])
                nc.scalar.activation(t[:], t[:], Act.Exp)
                if len(pending) >= LN_LAG:
                    emit_ln(*pending.pop(0))
                w = vpool.tile([P, ch // 2], f32, tag="pair")
                pair_combine_z(w[:], t[:, : ch // 2], t[:, ch // 2 :])
                if k in LEVEL2:
                    y = vpool.tile([P, ch // 4], f32, tag="pair2")
                    nc.gpsimd.tensor_mul(y[:], w[:, : ch // 4], w[:, ch // 4 :])
                    pending.append((y, k))
                else:
                    pending.append((w, k))
                off += ch
            for item in pending:
                emit_ln(*item)

            # ---------------- per-object terms ----------------
            # Everything consuming the gathered values is pinned late
            # (tile_wait_until) so no engine's in-order stream blocks on the
            # tiny gather DMAs before the big streaming work.
            with tc.tile_wait_until(0.06):
                sp = spool.tile([NOBJ, 1], f32)       # softplus(x_p)
                sptmp = spool.tile([NOBJ, 1], f32)
                softplus(sp[:], xp[:], tmp=sptmp[:])
                bce = spool.tile([NOBJ, 1], f32)      # positive-class bce
                nc.vector.tensor_sub(bce[:], sp[:], xp[:])

                d4 = spool.tile([NOBJ, 4], f32)
                nc.vector.tensor_sub(d4[:], bbp[:], T[:])
                l1r = spool.tile([NOBJ, 1], f32)
                nc.vector.tensor_reduce(
                    l1r[:], d4[:], axis=mybir.AxisListType.X, op=Alu.add,
                    apply_absolute_value=True,
                )

                M = spool.tile([NOBJ, 4], f32)  # [keep*bce, keep*sp, keep, l1/4]
                nc.vector.tensor_mul(M[:, 0:1], bce[:], keep[:])
                nc.vector.tensor_mul(M[:, 1:2], sp[:], keep[:])
                nc.vector.tensor_copy(M[:, 2:3], keep[:])
                nc.vector.tensor_scalar_mul(M[:, 3:4], l1r[:], 0.25)

                Fp = ppool.tile([2, 4], f32, space="PSUM")
                nc.tensor.matmul(Fp[:], bsel64[:], M[:], start=True, stop=True)
                F = spool.tile([2, 4], f32)           # per-image A, C, cnt, bbox
                nc.vector.tensor_copy(F[:], Fp[:])

            s0col = spool.tile([P, 1], f32)
            nc.vector.tensor_reduce(
                s0col[:], acc[:], axis=mybir.AxisListType.X, op=Alu.add
            )
            S0p = ppool.tile([2, 1], f32, space="PSUM")
            nc.tensor.matmul(S0p[:], bsel128[:], s0col[:], start=True, stop=True)

            # ---------------- per-image normalization ----------------
            negsum = spool.tile([2, 1], f32)          # S0 - sum(keep*sp)
            nc.vector.tensor_sub(negsum[:], S0p[:], F[:, 1:2])
            negcnt = spool.tile([2, 1], f32)          # CHW - cnt
            nc.vector.tensor_scalar(
                out=negcnt[:], in0=F[:, 2:3], scalar1=-1.0, scalar2=float(CHW),
                op0=Alu.mult, op1=Alu.add,
            )
            rc = spool.tile([2, 1], f32)
            nc.vector.reciprocal(rc[:], F[:, 2:3])
            rn = spool.tile([2, 1], f32)
            nc.vector.reciprocal(rn[:], negcnt[:])

            G = spool.tile([2, 3], f32)  # [pos_ratio, neg_ratio, bbox_sum]
            nc.vector.tensor_mul(G[:, 0:1], F[:, 0:1], rc[:])
            nc.vector.tensor_mul(G[:, 1:2], negsum[:], rn[:])
            nc.vector.tensor_copy(G[:, 2:3], F[:, 3:4])

            Rp = ppool.tile([1, 3], f32, space="PSUM")
            nc.tensor.matmul(Rp[:], ones2[:], G[:], start=True, stop=True)
            r = spool.tile([1, 3], f32)
            nc.vector.tensor_copy(r[:], Rp[:])

            # ---------------- final scalars (core's contribution) ----------------
            o = spool.tile([1, 3], f32)
            t2 = spool.tile([1, 1], f32)
            nc.vector.tensor_scalar_mul(t2[:], r[:, 1:2], 1.0 / (4 * B))
            # cls_i = pos_ratio/ (B*N) + neg_ratio/(4*B)
            nc.vector.scalar_tensor_tensor(
                out=o[:, 1:2], in0=r[:, 0:1], scalar=1.0 / (B * N),
                in1=t2[:], op0=Alu.mult, op1=Alu.add,
            )
            nc.vector.tensor_scalar_mul(o[:, 2:3], r[:, 2:3], 1.0 / (B * N))
            t3 = spool.tile([1, 1], f32)
            nc.vector.tensor_scalar(
                out=t3[:], in0=o[:, 2:3], scalar1=7.5, scalar2=1e-6 / N_CORES,
                op0=Alu.mult, op1=Alu.add,
            )
            nc.vector.scalar_tensor_tensor(
                out=o[:, 0:1], in0=o[:, 1:2], scalar=0.5,
                in1=t3[:], op0=Alu.mult, op1=Alu.add,
            )
            nc.sync.dma_start(out_d[:], o[:])


def _get_nc(repeat=1):
    if repeat not in _cache:
        _cache[repeat] = _build_nc(repeat)
    return _cache[repeat]


def _make_in_maps(cls_pred, bbox_pred, gt_bboxes, gt_labels):
    cls_pred = np.ascontiguousarray(np.asarray(cls_pred, dtype=np.float32))
    bbox_pred = np.ascontiguousarray(np.asarray(bbox_pred, dtype=np.float32))
    gt_bboxes = np.ascontiguousarray(np.asarray(gt_bboxes, dtype=np.float32))
    gt_labels = np.asarray(gt_labels).astype(np.int32)
    in_maps = []
    for i in range(N_CORES):
        s = slice(i * BS, (i + 1) * BS)
        in_maps.append(
            {
                "cls": cls_pred[s].reshape(CLS_FLAT),
                "bb": bbox_pred[s].reshape(BB_FLAT),
                "gt": gt_bboxes[s].reshape(NOBJ, 4),
                "lbl": np.ascontiguousarray(gt_labels[s].reshape(NOBJ, 1)),
            }
        )
    return in_maps


def kernel_with_results(trace=False, **inputs):
    from concourse.bass_utils import run_bass_kernel_spmd

    nc = _get_nc()
    in_maps = _make_in_maps(**inputs)
    res = run_bass_kernel_spmd(
        nc, in_maps, core_ids=list(range(N_CORES)), trace=trace
    )
    total = np.zeros(3, dtype=np.float64)
    for core in res.results:
        total += core["out"].reshape(3).astype(np.float64)
    return total.astype(np.float32), res


def kernel(**inputs):
    out, _ = kernel_with_results(**inputs)
    return out
